# revision 1
# baseline (speedup 1.0000x reference)
"""Trainium2 Bass kernel for nn_Net_34359738709 (spiking RNN).

Model (per timestep t, reference semantics):
    cur1  = x_t @ W1.T + b1                      # [B, NH] big matmul, t-independent
    mem1  = beta1c*mem1 + cur1 + spk1 @ V.T + Vb - spk1*THRESH
    spk1  = (mem1 - THRESH > 0)
    cur2  = spk1 @ W2.T + b2
    mem2  = BETA2*mem2 + cur2 - spk2_prev*THRESH
    spk2  = (mem2 - THRESH > 0)
outputs: (spk2_rec, mem2_rec), each [T, B, NOUT]

Strategy: data-parallel over batch (B=64 -> 8 cores x 8). The x @ W1.T
matmul (21 GFLOP) is hoisted out of the time scan and computed as
cur1.T[NH, T*BL] = W1 @ x.T, accumulated over 256 K-chunks of 128 in a
single PSUM bank per column tile. Column tiles ([136,136,128] cols = 17/17/16
timesteps) let each tile's tiny sequential scan overlap the next tile's
matmuls/DMA. The recurrent term spk1@V.T + Vb - spk1 is one augmented
128-contraction matmul per step: lhsT rows 0..99 = (V-I).T, row 100 = Vb,
rhs = [spk1; 1; 0-pad].
"""

import sys

if "/opt/trn_rl_repo" not in sys.path:
    sys.path.insert(0, "/opt/trn_rl_repo")

import numpy as np

# Problem shapes (hardcoded per contract)
T, B, NIN, NH, NOUT = 50, 64, 32768, 100, 11
NCORES = 8
BL = B // NCORES          # 8 batch rows per core
TBL = T * BL              # 400 columns (t-major: col = t*BL + b)
KP = 128                  # contraction partition size
KCH = NIN // KP           # 256 K-chunks
CG = 16                   # K-chunks per x DMA group (~1.1 MB per dma_start)
COL_TILES = [(0, 136), (136, 136), (272, 128)]  # (col0, ncols), ncols % BL == 0
THRESH = 1.0
BETA2 = 0.9753

_PROG = None  # cached compiled Bass program


def _build_body(tc, nc, mybir, aps):
    import concourse.tile as tile  # noqa: F401

    f32 = mybir.dt.float32
    Alu = mybir.AluOpType
    xT, w1t, a1, w2a, b1, bet, s1init, spk_o, mem_o = aps

    ctx = tc.ctx if hasattr(tc, "ctx") else None
    # pools
    from contextlib import ExitStack

    stack = ExitStack()
    const_pool = stack.enter_context(tc.tile_pool(name="const", bufs=1))
    state_pool = stack.enter_context(tc.tile_pool(name="state", bufs=1))
    xpool = stack.enter_context(tc.tile_pool(name="xg", bufs=3))
    curpool = stack.enter_context(tc.tile_pool(name="cur", bufs=2))
    ps_big = stack.enter_context(tc.tile_pool(name="psbig", bufs=2, space="PSUM"))
    ps_s1 = stack.enter_context(tc.tile_pool(name="pss1", bufs=2, space="PSUM"))
    ps_s2 = stack.enter_context(tc.tile_pool(name="pss2", bufs=2, space="PSUM"))

    MAXC = max(c for _, c in COL_TILES)

    # ---- constants into SBUF ----
    w1sb = const_pool.tile([KP, KCH, NH], f32)       # 12.8 MB resident
    w1v = w1t.rearrange("(c p) m -> p c m", p=KP)    # [128, 256, 100]
    NW = 8
    for g in range(NW):
        gs = KCH // NW
        nc.sync.dma_start(w1sb[:, g * gs:(g + 1) * gs, :], w1v[:, g * gs:(g + 1) * gs, :])
    a1sb = const_pool.tile([KP, NH], f32)
    nc.sync.dma_start(a1sb[:], a1)
    w2sb = const_pool.tile([KP, NOUT], f32)
    nc.sync.dma_start(w2sb[:], w2a)
    b1sb = const_pool.tile([NH, 1], f32)
    nc.sync.dma_start(b1sb[:], b1)
    betsb = const_pool.tile([NH, 1], f32)
    nc.sync.dma_start(betsb[:], bet)

    # ---- state ----
    spk1aug = state_pool.tile([KP, BL], f32)   # rows 0..99 spk1, row 100 = 1, rest 0
    nc.sync.dma_start(spk1aug[:], s1init)
    mem1 = state_pool.tile([NH, BL], f32)
    nc.vector.memset(mem1[:], 0.0)
    m2rec = state_pool.tile([NOUT, BL + TBL], f32)   # col block t+1 = mem2 at step t
    s2rec = state_pool.tile([NOUT, BL + TBL], f32)
    nc.vector.memset(m2rec[:, 0:BL], 0.0)
    nc.vector.memset(s2rec[:, 0:BL], 0.0)

    xv = xT.rearrange("(c p) n -> p c n", p=KP)      # [128, 256, 400]

    t_global = 0
    for (col0, cols) in COL_TILES:
        ps = ps_big.tile([NH, MAXC], f32)
        for cg in range(KCH // CG):
            xg = xpool.tile([KP, CG, MAXC], f32)
            nc.sync.dma_start(
                xg[:, :, :cols],
                xv[:, cg * CG:(cg + 1) * CG, col0:col0 + cols],
            )
            for ci in range(CG):
                c = cg * CG + ci
                nc.tensor.matmul(
                    ps[:, :cols],
                    lhsT=w1sb[:, c, :],
                    rhs=xg[:, ci, :cols],
                    start=(c == 0),
                    stop=(c == KCH - 1),
                )
        cur = curpool.tile([NH, MAXC], f32)
        nc.vector.tensor_scalar_add(cur[:, :cols], ps[:, :cols], b1sb[:, 0:1])

        # ---- sequential scan for this tile's timesteps ----
        for k in range(cols // BL):
            t = t_global + k
            lc = k * BL
            rec = ps_s1.tile([NH, BL], f32)
            nc.tensor.matmul(rec[:], lhsT=a1sb[:, :], rhs=spk1aug[:, :],
                             start=True, stop=True)
            # mem1 = mem1*beta + cur1_t
            nc.vector.scalar_tensor_tensor(
                mem1[:], mem1[:], betsb[:, 0:1], cur[:, lc:lc + BL],
                Alu.mult, Alu.add)
            # mem1 += (V-I)@spk1 + Vb
            nc.vector.tensor_add(mem1[:], mem1[:], rec[:])
            # spk1 = mem1 > 1
            nc.vector.tensor_scalar(spk1aug[0:NH, :], mem1[:], THRESH, None, Alu.is_gt)
            o2 = ps_s2.tile([NOUT, BL], f32)
            nc.tensor.matmul(o2[:], lhsT=w2sb[:, :], rhs=spk1aug[:, :],
                             start=True, stop=True)
            mprev = m2rec[:, t * BL:(t + 1) * BL]
            mcur = m2rec[:, (t + 1) * BL:(t + 2) * BL]
            sprev = s2rec[:, t * BL:(t + 1) * BL]
            scur = s2rec[:, (t + 1) * BL:(t + 2) * BL]
            # mem2 = mem2_prev*BETA2 - spk2_prev
            nc.vector.scalar_tensor_tensor(
                mcur, mprev, BETA2, sprev, Alu.mult, Alu.subtract)
            # mem2 += W2@spk1 + b2
            nc.vector.tensor_add(mcur, mcur, o2[:])
            # spk2 = mem2 > 1
            nc.vector.tensor_scalar(scur, mcur, THRESH, None, Alu.is_gt)
        t_global += cols // BL

    nc.sync.dma_start(spk_o[:], s2rec[:, BL:BL + TBL])
    nc.sync.dma_start(mem_o[:], m2rec[:, BL:BL + TBL])
    stack.close()


def build_program():
    global _PROG
    if _PROG is not None:
        return _PROG
    import concourse.tile as tile
    from concourse import bacc, mybir

    f32 = mybir.dt.float32
    nc = bacc.Bacc("TRN2", target_bir_lowering=False, debug=False,
                   num_devices=NCORES)
    xT = nc.dram_tensor("xT", [NIN, TBL], f32, kind="ExternalInput").ap()
    w1t = nc.dram_tensor("w1t", [NIN, NH], f32, kind="ExternalInput").ap()
    a1 = nc.dram_tensor("a1", [KP, NH], f32, kind="ExternalInput").ap()
    w2a = nc.dram_tensor("w2a", [KP, NOUT], f32, kind="ExternalInput").ap()
    b1 = nc.dram_tensor("b1", [NH, 1], f32, kind="ExternalInput").ap()
    bet = nc.dram_tensor("bet", [NH, 1], f32, kind="ExternalInput").ap()
    s1init = nc.dram_tensor("s1init", [KP, BL], f32, kind="ExternalInput").ap()
    spk_o = nc.dram_tensor("spk", [NOUT, TBL], f32, kind="ExternalOutput").ap()
    mem_o = nc.dram_tensor("mem", [NOUT, TBL], f32, kind="ExternalOutput").ap()
    aps = (xT, w1t, a1, w2a, b1, bet, s1init, spk_o, mem_o)
    with tile.TileContext(nc) as tc:
        _build_body(tc, nc, mybir, aps)
    nc.compile()
    _PROG = nc
    return nc


def prep_inputs(x, W1, b1, beta1, V, Vb, W2, b2):
    """Host-side shard + layout prep. Returns list of per-core input dicts."""
    f32 = np.float32
    w1t = np.ascontiguousarray(W1.T, dtype=f32)                 # [NIN, NH]
    a1 = np.zeros((KP, NH), f32)
    a1[:NH] = (V - THRESH * np.eye(NH, dtype=f32)).T
    a1[NH] = Vb
    w2a = np.zeros((KP, NOUT), f32)
    w2a[:NH] = W2.T
    w2a[NH] = b2
    b1a = np.ascontiguousarray(b1.reshape(NH, 1), dtype=f32)
    beta = np.clip(beta1, 0.0, 1.0).astype(f32).reshape(NH, 1)
    s1init = np.zeros((KP, BL), f32)
    s1init[NH] = 1.0
    # x: [T, B, NIN] -> per-core [NIN, T*BL] (t-major cols)
    xt_full = np.ascontiguousarray(x.transpose(2, 0, 1))        # [NIN, T, B]
    in_maps = []
    for c in range(NCORES):
        xT = np.ascontiguousarray(xt_full[:, :, c * BL:(c + 1) * BL]).reshape(NIN, TBL)
        in_maps.append(dict(xT=xT, w1t=w1t, a1=a1, w2a=w2a, b1=b1a,
                            bet=beta, s1init=s1init))
    return in_maps


def gather_outputs(results):
    """results: list of per-core {'spk': [NOUT, TBL], 'mem': [NOUT, TBL]}."""
    spks, mems = [], []
    for r in results:
        # [NOUT, T, BL] -> [T, BL, NOUT]
        spks.append(np.ascontiguousarray(
            r["spk"].reshape(NOUT, T, BL).transpose(1, 2, 0)))
        mems.append(np.ascontiguousarray(
            r["mem"].reshape(NOUT, T, BL).transpose(1, 2, 0)))
    spk = np.concatenate(spks, axis=1)
    mem = np.concatenate(mems, axis=1)
    return spk.astype(np.float32), mem.astype(np.float32)


def kernel(x, W1, b1, beta1, V, Vb, W2, b2, **_run_kwargs):
    from concourse import bass_utils

    x = np.asarray(x, np.float32)
    nc = build_program()
    in_maps = prep_inputs(np.asarray(x, np.float32), np.asarray(W1, np.float32),
                          np.asarray(b1, np.float32), np.asarray(beta1, np.float32),
                          np.asarray(V, np.float32), np.asarray(Vb, np.float32),
                          np.asarray(W2, np.float32), np.asarray(b2, np.float32))
    res = bass_utils.run_bass_kernel_spmd(
        nc, in_maps, core_ids=list(range(NCORES)), **_run_kwargs)
    out = gather_outputs(res.results)
    kernel.last_result = res
    return out


# revision 2
# speedup vs baseline: 1.1038x; 1.1038x over previous
"""Trainium2 Bass kernel for nn_Net_34359738709 (spiking RNN).

Model (per timestep t, reference semantics):
    cur1  = x_t @ W1.T + b1                      # [B, NH] big matmul, t-independent
    mem1  = beta1c*mem1 + cur1 + spk1 @ V.T + Vb - spk1*THRESH
    spk1  = (mem1 - THRESH > 0)
    cur2  = spk1 @ W2.T + b2
    mem2  = BETA2*mem2 + cur2 - spk2_prev*THRESH
    spk2  = (mem2 - THRESH > 0)
outputs: (spk2_rec, mem2_rec), each [T, B, NOUT]

Strategy: data-parallel over batch (B=64 -> 8 cores x 8). The x @ W1.T
matmul (21 GFLOP) is hoisted out of the time scan and computed as
cur1.T[NH, T*BL] = W1 @ x.T, accumulated over 256 K-chunks of 128 in one
PSUM bank per column tile. Column tiles (17/17/16 timesteps) let each
tile's tiny sequential scan overlap the next tile's matmuls/DMA. The
recurrent term spk1@V.T + Vb - spk1 is one augmented 128-contraction
matmul per step: lhsT rows 0..99 = (V-I).T, row 100 = Vb, rhs =
[spk1; 1; 0-pad]. Inputs are host-pre-arranged into the exact SBUF
layouts so every DMA is long-contiguous per partition (line rate).
"""

import sys

if "/opt/trn_rl_repo" not in sys.path:
    sys.path.insert(0, "/opt/trn_rl_repo")

import numpy as np

# Problem shapes (hardcoded per contract)
T, B, NIN, NH, NOUT = 50, 64, 32768, 100, 11
NCORES = 8
BL = B // NCORES          # 8 batch rows per core
TBL = T * BL              # 400 columns (t-major: col = t*BL + b)
KP = 128                  # contraction partition size
KCH = NIN // KP           # 256 K-chunks
CG = 16                   # K-chunks per x DMA group
COL_TILES = [136, 136, 128]   # ncols per column tile, each % BL == 0
THRESH = 1.0
BETA2 = 0.9753

PRECISION = "fp32"        # "fp32" | "fp32r" | "bf16x2"

_PROG = {}


def _build_body(tc, nc, mybir, aps, precision):
    f32 = mybir.dt.float32
    Alu = mybir.AluOpType
    mm_dt = {"fp32": f32, "fp32r": mybir.dt.float32r,
             "bf16x2": mybir.dt.bfloat16}[precision]
    NSPLIT = 2 if precision == "bf16x2" else 1
    xts, w1f, a1, w2a, b1, bet, s1init, spk_o, mem_o = aps

    from contextlib import ExitStack

    stack = ExitStack()
    const_pool = stack.enter_context(tc.tile_pool(name="const", bufs=1))
    state_pool = stack.enter_context(tc.tile_pool(name="state", bufs=1))
    xpool = stack.enter_context(tc.tile_pool(name="xg", bufs=3))
    curpool = stack.enter_context(tc.tile_pool(name="cur", bufs=2))
    ps_big = stack.enter_context(tc.tile_pool(name="psbig", bufs=2, space="PSUM"))
    ps_s1 = stack.enter_context(tc.tile_pool(name="pss1", bufs=2, space="PSUM"))
    ps_s2 = stack.enter_context(tc.tile_pool(name="pss2", bufs=2, space="PSUM"))

    MAXC = max(COL_TILES)

    # ---- small constants (scalar-engine HWDGE ring; ahead of W1 groups) ----
    a1sb = const_pool.tile([KP, NH], f32)
    nc.scalar.dma_start(a1sb[:], a1)
    w2sb = const_pool.tile([KP, NOUT], f32)
    nc.scalar.dma_start(w2sb[:], w2a)
    b1sb = const_pool.tile([NH, 1], f32)
    nc.scalar.dma_start(b1sb[:], b1)
    betsb = const_pool.tile([NH, 1], f32)
    nc.scalar.dma_start(betsb[:], bet)
    spk1aug = state_pool.tile([KP, BL], f32)
    nc.scalar.dma_start(spk1aug[:], s1init)

    # W1 resident in SBUF, exact matmul layout [128, NSPLIT*KCH*NH]
    w1sb = const_pool.tile([KP, NSPLIT * KCH * NH], mm_dt)
    NWG = KCH // CG  # W1 DMA groups, interleaved with tile-0 x groups below

    def w1_chunk(c, s=0):
        base = (s * KCH + c) * NH
        return w1sb[:, base:base + NH]

    # ---- state ----
    mem1 = state_pool.tile([NH, BL], f32)
    nc.vector.memset(mem1[:], 0.0)
    m2rec = state_pool.tile([NOUT, BL + TBL], f32)
    s2rec = state_pool.tile([NOUT, BL + TBL], f32)
    nc.vector.memset(m2rec[:, 0:BL], 0.0)
    nc.vector.memset(s2rec[:, 0:BL], 0.0)

    t_global = 0
    for j, cols in enumerate(COL_TILES):
        xt = xts[j]           # [128, NSPLIT*KCH*cols] dram, matmul-ready
        ps = ps_big.tile([NH, MAXC], f32)
        for cg in range(KCH // CG):
            if j == 0 and cg < NWG:
                gsz = NSPLIT * (KCH // NWG) * NH
                nc.scalar.dma_start(
                    w1sb[:, cg * gsz:(cg + 1) * gsz],
                    w1f[:, cg * gsz:(cg + 1) * gsz])
            xg = xpool.tile([KP, NSPLIT * CG * MAXC], mm_dt)
            gsz = NSPLIT * CG * cols
            nc.sync.dma_start(xg[:, :gsz], xt[:, cg * gsz:(cg + 1) * gsz])

            def xg_chunk(ci, s=0):
                base = (ci * NSPLIT + s) * cols
                return xg[:, base:base + cols]

            for ci in range(CG):
                c = cg * CG + ci
                if NSPLIT == 1:
                    nc.tensor.matmul(
                        ps[:, :cols], lhsT=w1_chunk(c), rhs=xg_chunk(ci),
                        start=(c == 0), stop=(c == KCH - 1))
                else:
                    # hi/lo split: hh + hl + lh (ll dropped, ~2^-17 rel)
                    nc.tensor.matmul(
                        ps[:, :cols], lhsT=w1_chunk(c, 0), rhs=xg_chunk(ci, 0),
                        start=(c == 0), stop=False)
                    nc.tensor.matmul(
                        ps[:, :cols], lhsT=w1_chunk(c, 1), rhs=xg_chunk(ci, 0),
                        start=False, stop=False)
                    nc.tensor.matmul(
                        ps[:, :cols], lhsT=w1_chunk(c, 0), rhs=xg_chunk(ci, 1),
                        start=False, stop=(c == KCH - 1))
        cur = curpool.tile([NH, MAXC], f32)
        nc.vector.tensor_scalar_add(cur[:, :cols], ps[:, :cols], b1sb[:, 0:1])

        # ---- sequential scan for this tile's timesteps ----
        for k in range(cols // BL):
            t = t_global + k
            lc = k * BL
            rec = ps_s1.tile([NH, BL], f32)
            nc.tensor.matmul(rec[:], lhsT=a1sb[:, :], rhs=spk1aug[:, :],
                             start=True, stop=True)
            nc.vector.scalar_tensor_tensor(
                mem1[:], mem1[:], betsb[:, 0:1], cur[:, lc:lc + BL],
                Alu.mult, Alu.add)
            nc.vector.tensor_add(mem1[:], mem1[:], rec[:])
            nc.vector.tensor_scalar(spk1aug[0:NH, :], mem1[:], THRESH, None, Alu.is_gt)
            o2 = ps_s2.tile([NOUT, BL], f32)
            nc.tensor.matmul(o2[:], lhsT=w2sb[:, :], rhs=spk1aug[:, :],
                             start=True, stop=True)
            mprev = m2rec[:, t * BL:(t + 1) * BL]
            mcur = m2rec[:, (t + 1) * BL:(t + 2) * BL]
            sprev = s2rec[:, t * BL:(t + 1) * BL]
            scur = s2rec[:, (t + 1) * BL:(t + 2) * BL]
            nc.vector.scalar_tensor_tensor(
                mcur, mprev, BETA2, sprev, Alu.mult, Alu.subtract)
            nc.vector.tensor_add(mcur, mcur, o2[:])
            nc.vector.tensor_scalar(scur, mcur, THRESH, None, Alu.is_gt)
        t_global += cols // BL

    nc.sync.dma_start(spk_o[:], s2rec[:, BL:BL + TBL])
    nc.sync.dma_start(mem_o[:], m2rec[:, BL:BL + TBL])
    stack.close()


def build_program(precision=None):
    precision = precision or PRECISION
    if precision in _PROG:
        return _PROG[precision]
    import concourse.tile as tile
    from concourse import bacc, mybir

    f32 = mybir.dt.float32
    mm_dt = {"fp32": f32, "fp32r": mybir.dt.float32r,
             "bf16x2": mybir.dt.bfloat16}[precision]
    NSPLIT = 2 if precision == "bf16x2" else 1
    nc = bacc.Bacc("TRN2", target_bir_lowering=False, debug=False,
                   num_devices=NCORES)
    xts = [nc.dram_tensor(f"xt{j}", [KP, NSPLIT * KCH * cols], mm_dt,
                          kind="ExternalInput").ap()
           for j, cols in enumerate(COL_TILES)]
    w1f = nc.dram_tensor("w1f", [KP, NSPLIT * KCH * NH], mm_dt,
                         kind="ExternalInput").ap()
    a1 = nc.dram_tensor("a1", [KP, NH], f32, kind="ExternalInput").ap()
    w2a = nc.dram_tensor("w2a", [KP, NOUT], f32, kind="ExternalInput").ap()
    b1 = nc.dram_tensor("b1", [NH, 1], f32, kind="ExternalInput").ap()
    bet = nc.dram_tensor("bet", [NH, 1], f32, kind="ExternalInput").ap()
    s1init = nc.dram_tensor("s1init", [KP, BL], f32, kind="ExternalInput").ap()
    spk_o = nc.dram_tensor("spk", [NOUT, TBL], f32, kind="ExternalOutput").ap()
    mem_o = nc.dram_tensor("mem", [NOUT, TBL], f32, kind="ExternalOutput").ap()
    aps = (xts, w1f, a1, w2a, b1, bet, s1init, spk_o, mem_o)
    with tile.TileContext(nc) as tc:
        _build_body(tc, nc, mybir, aps, precision)
    nc.compile()
    _PROG[precision] = nc
    return nc


def _split_hi_lo(a):
    import ml_dtypes
    hi = a.astype(ml_dtypes.bfloat16)
    lo = (a - hi.astype(np.float32)).astype(ml_dtypes.bfloat16)
    return hi, lo


def _mm_layout(kxn, nsplit):
    """[K=NIN, N] fp32 -> [128, nsplit*KCH*N] in matmul-ready order."""
    n = kxn.shape[1]
    v = np.ascontiguousarray(
        kxn.reshape(KCH, KP, n).transpose(1, 0, 2))     # [128, KCH, n]
    if nsplit == 1:
        return v.reshape(KP, KCH * n)
    hi, lo = _split_hi_lo(v)
    out = np.empty((KP, KCH, 2, n), hi.dtype)
    out[:, :, 0, :] = hi
    out[:, :, 1, :] = lo
    return np.ascontiguousarray(out).reshape(KP, 2 * KCH * n)


def prep_inputs(x, W1, b1, beta1, V, Vb, W2, b2, precision=None):
    """Host-side shard + layout prep. Returns list of per-core input dicts."""
    precision = precision or PRECISION
    nsplit = 2 if precision == "bf16x2" else 1
    f32 = np.float32
    w1f = _mm_layout(np.ascontiguousarray(W1.T, dtype=f32), nsplit)
    a1 = np.zeros((KP, NH), f32)
    a1[:NH] = (V - THRESH * np.eye(NH, dtype=f32)).T
    a1[NH] = Vb
    w2a = np.zeros((KP, NOUT), f32)
    w2a[:NH] = W2.T
    w2a[NH] = b2
    b1a = np.ascontiguousarray(b1.reshape(NH, 1), dtype=f32)
    beta = np.clip(beta1, 0.0, 1.0).astype(f32).reshape(NH, 1)
    s1init = np.zeros((KP, BL), f32)
    s1init[NH] = 1.0
    # x: [T, B, NIN] -> per-core column tiles in matmul-ready layout
    xt_full = np.ascontiguousarray(x.transpose(2, 0, 1))        # [NIN, T, B]
    col_edges = np.cumsum([0] + COL_TILES)
    in_maps = []
    for c in range(NCORES):
        xTc = np.ascontiguousarray(
            xt_full[:, :, c * BL:(c + 1) * BL]).reshape(NIN, TBL)
        m = dict(w1f=w1f, a1=a1, w2a=w2a, b1=b1a, bet=beta, s1init=s1init)
        for j, cols in enumerate(COL_TILES):
            m[f"xt{j}"] = _mm_layout(
                np.ascontiguousarray(xTc[:, col_edges[j]:col_edges[j + 1]]),
                nsplit)
        in_maps.append(m)
    return in_maps


def gather_outputs(results):
    """results: list of per-core {'spk': [NOUT, TBL], 'mem': [NOUT, TBL]}."""
    spks, mems = [], []
    for r in results:
        spks.append(np.ascontiguousarray(
            r["spk"].reshape(NOUT, T, BL).transpose(1, 2, 0)))
        mems.append(np.ascontiguousarray(
            r["mem"].reshape(NOUT, T, BL).transpose(1, 2, 0)))
    spk = np.concatenate(spks, axis=1)
    mem = np.concatenate(mems, axis=1)
    return spk.astype(np.float32), mem.astype(np.float32)


def kernel(x, W1, b1, beta1, V, Vb, W2, b2, **_run_kwargs):
    from concourse import bass_utils

    precision = _run_kwargs.pop("precision", None) or PRECISION
    nc = build_program(precision)
    in_maps = prep_inputs(np.asarray(x, np.float32), np.asarray(W1, np.float32),
                          np.asarray(b1, np.float32), np.asarray(beta1, np.float32),
                          np.asarray(V, np.float32), np.asarray(Vb, np.float32),
                          np.asarray(W2, np.float32), np.asarray(b2, np.float32),
                          precision)
    res = bass_utils.run_bass_kernel_spmd(
        nc, in_maps, core_ids=list(range(NCORES)), **_run_kwargs)
    out = gather_outputs(res.results)
    kernel.last_result = res
    return out


# revision 3
# speedup vs baseline: 1.2437x; 1.1267x over previous
"""Trainium2 Bass kernel for nn_Net_34359738709 (spiking RNN).

Model (per timestep t, reference semantics):
    cur1  = x_t @ W1.T + b1                      # [B, NH] big matmul, t-independent
    mem1  = beta1c*mem1 + cur1 + spk1 @ V.T + Vb - spk1*THRESH
    spk1  = (mem1 - THRESH > 0)
    cur2  = spk1 @ W2.T + b2
    mem2  = BETA2*mem2 + cur2 - spk2_prev*THRESH
    spk2  = (mem2 - THRESH > 0)
outputs: (spk2_rec, mem2_rec), each [T, B, NOUT]

Strategy: data-parallel over batch (B=64 -> 8 cores x 8). The x @ W1.T
matmul (21 GFLOP) is hoisted out of the time scan and computed as
cur1.T[NH, T*BL] = W1 @ x.T, accumulated over 256 K-chunks of 128 in one
PSUM bank per column tile. fp32 accuracy at bf16 speed via a hi/lo split:
x = x_hi + x_lo, W1 = w_hi + w_lo (each bf16), cur1 ~= w_hi@x_hi +
w_hi@x_lo + w_lo@x_hi (verified exact spike pattern vs fp32). Two column
tiles (34/16 timesteps): the first tile's sequential scan overlaps the
second tile's matmuls. Per scan step, layer 1 runs one augmented
128-contraction matmul (lhsT rows 0..99 = (V-I).T, row 100 = Vb, rhs =
[spk1; 1; 0]) plus three vector ops; layer 2 uses a per-burst batched
W2 matmul then a vector-only 3-op chain per step. All inputs are
host-pre-arranged into exact SBUF layouts so DMA runs long-contiguous.
"""

import sys

if "/opt/trn_rl_repo" not in sys.path:
    sys.path.insert(0, "/opt/trn_rl_repo")

import numpy as np

# Problem shapes (hardcoded per contract)
T, B, NIN, NH, NOUT = 50, 64, 32768, 100, 11
NCORES = 8
BL = B // NCORES          # 8 batch rows per core
TBL = T * BL              # 400 columns (t-major: col = t*BL + b)
KP = 128                  # contraction partition size
KCH = NIN // KP           # 256 K-chunks
COL_TILES = [272, 128]    # ncols per column tile, each % BL == 0
X_GROUPS = [4, 4] + [8] * 31   # K-chunks per x dma_start (sums to 256)
THRESH = 1.0
BETA2 = 0.9753

PRECISION = "bf16x2"      # "fp32" | "bf16x2"

_PROG = {}


def _build_body(tc, nc, mybir, aps, precision):
    f32 = mybir.dt.float32
    Alu = mybir.AluOpType
    mm_dt = {"fp32": f32, "bf16x2": mybir.dt.bfloat16}[precision]
    NS = 2 if precision == "bf16x2" else 1
    xts, w1f, a1, w2a, b1, bet, s1init, spk_o, mem_o = aps

    from contextlib import ExitStack

    stack = ExitStack()
    const_pool = stack.enter_context(tc.tile_pool(name="const", bufs=1))
    state_pool = stack.enter_context(tc.tile_pool(name="state", bufs=1))
    xpool = stack.enter_context(tc.tile_pool(name="xg", bufs=5))
    curpool = stack.enter_context(tc.tile_pool(name="cur", bufs=2))
    ps_big = stack.enter_context(tc.tile_pool(name="psbig", bufs=2, space="PSUM"))
    ps_s1 = stack.enter_context(tc.tile_pool(name="pss1", bufs=2, space="PSUM"))
    ps_c2 = stack.enter_context(tc.tile_pool(name="psc2", bufs=2, space="PSUM"))

    MAXC = max(COL_TILES)
    MAXG = max(X_GROUPS)

    # ---- small constants (scalar-engine HWDGE ring, ahead of W1 groups) ----
    a1sb = const_pool.tile([KP, NH], f32)
    nc.scalar.dma_start(a1sb[:], a1)
    w2sb = const_pool.tile([KP, NOUT], f32)
    nc.scalar.dma_start(w2sb[:], w2a)
    b1sb = const_pool.tile([NH, 1], f32)
    nc.scalar.dma_start(b1sb[:], b1)
    betsb = const_pool.tile([NH, 1], f32)
    nc.scalar.dma_start(betsb[:], bet)
    # spk1 ring buffer: col block t+1 = spk1 after step t; rows 100..127
    # carry the [1; 0-pad] augmentation for every column (from s1init).
    spk1buf = state_pool.tile([KP, BL + TBL], f32)
    nc.scalar.dma_start(spk1buf[:], s1init)

    # W1 resident in SBUF, exact matmul layout [128, NS*KCH*NH] (bf16 hi|lo)
    w1sb = const_pool.tile([KP, NS * KCH * NH], mm_dt)

    def w1_chunk(c, s=0):
        base = (c * NS + s) * NH
        return w1sb[:, base:base + NH]

    # ---- state ----
    mem1 = state_pool.tile([NH, BL], f32)
    nc.vector.memset(mem1[:], 0.0)
    m2rec = state_pool.tile([NOUT, BL + TBL], f32)
    s2rec = state_pool.tile([NOUT, BL + TBL], f32)
    nc.vector.memset(m2rec[:, 0:BL], 0.0)
    nc.vector.memset(s2rec[:, 0:BL], 0.0)

    t_global = 0
    for j, cols in enumerate(COL_TILES):
        xt = xts[j]           # [128, NS*KCH*cols] dram, matmul-ready
        ps = ps_big.tile([NH, MAXC], f32)
        c0 = 0
        for g, gch in enumerate(X_GROUPS):
            if j == 0:
                # stream the matching W1 chunk range on the scalar ring
                w0, w1n = c0 * NS * NH, (c0 + gch) * NS * NH
                nc.scalar.dma_start(w1sb[:, w0:w1n], w1f[:, w0:w1n])
            xg = xpool.tile([KP, NS * MAXG * MAXC], mm_dt)
            gsz = NS * gch * cols
            nc.sync.dma_start(xg[:, :gsz], xt[:, c0 * NS * cols:(c0 + gch) * NS * cols])

            def xg_chunk(ci, s=0):
                base = (ci * NS + s) * cols
                return xg[:, base:base + cols]

            for ci in range(gch):
                c = c0 + ci
                if NS == 1:
                    nc.tensor.matmul(
                        ps[:, :cols], lhsT=w1_chunk(c), rhs=xg_chunk(ci),
                        start=(c == 0), stop=(c == KCH - 1))
                else:
                    # hi/lo split: hh + hl + lh (ll dropped, ~2^-17 rel)
                    nc.tensor.matmul(
                        ps[:, :cols], lhsT=w1_chunk(c, 0), rhs=xg_chunk(ci, 0),
                        start=(c == 0), stop=False)
                    nc.tensor.matmul(
                        ps[:, :cols], lhsT=w1_chunk(c, 0), rhs=xg_chunk(ci, 1),
                        start=False, stop=False)
                    nc.tensor.matmul(
                        ps[:, :cols], lhsT=w1_chunk(c, 1), rhs=xg_chunk(ci, 0),
                        start=False, stop=(c == KCH - 1))
            c0 += gch
        cur = curpool.tile([NH, MAXC], f32)
        nc.vector.tensor_scalar_add(cur[:, :cols], ps[:, :cols], b1sb[:, 0:1])

        # ---- layer-1 sequential scan for this tile's timesteps ----
        nsteps = cols // BL
        for k in range(nsteps):
            t = t_global + k
            rec = ps_s1.tile([NH, BL], f32)
            nc.tensor.matmul(rec[:], lhsT=a1sb[:, :],
                             rhs=spk1buf[:, t * BL:(t + 1) * BL],
                             start=True, stop=True)
            nc.vector.scalar_tensor_tensor(
                mem1[:], mem1[:], betsb[:, 0:1], cur[:, k * BL:(k + 1) * BL],
                Alu.mult, Alu.add)
            nc.vector.tensor_add(mem1[:], mem1[:], rec[:])
            nc.vector.tensor_scalar(
                spk1buf[0:NH, (t + 1) * BL:(t + 2) * BL], mem1[:],
                THRESH, None, Alu.is_gt)

        # ---- layer 2: one batched matmul, then vector-only chain ----
        c2 = ps_c2.tile([NOUT, MAXC], f32)
        nc.tensor.matmul(c2[:, :cols], lhsT=w2sb[:, :],
                         rhs=spk1buf[:, (t_global + 1) * BL:
                                     (t_global + 1 + nsteps) * BL],
                         start=True, stop=True)
        for k in range(nsteps):
            t = t_global + k
            mprev = m2rec[:, t * BL:(t + 1) * BL]
            mcur = m2rec[:, (t + 1) * BL:(t + 2) * BL]
            sprev = s2rec[:, t * BL:(t + 1) * BL]
            scur = s2rec[:, (t + 1) * BL:(t + 2) * BL]
            nc.vector.scalar_tensor_tensor(
                mcur, mprev, BETA2, sprev, Alu.mult, Alu.subtract)
            nc.vector.tensor_add(mcur, mcur, c2[:, k * BL:(k + 1) * BL])
            nc.vector.tensor_scalar(scur, mcur, THRESH, None, Alu.is_gt)
        t_global += nsteps

    nc.sync.dma_start(spk_o[:], s2rec[:, BL:BL + TBL])
    nc.sync.dma_start(mem_o[:], m2rec[:, BL:BL + TBL])
    stack.close()


def build_program(precision=None):
    precision = precision or PRECISION
    if precision in _PROG:
        return _PROG[precision]
    import concourse.tile as tile
    from concourse import bacc, mybir

    f32 = mybir.dt.float32
    mm_dt = {"fp32": f32, "bf16x2": mybir.dt.bfloat16}[precision]
    NS = 2 if precision == "bf16x2" else 1
    nc = bacc.Bacc("TRN2", target_bir_lowering=False, debug=False,
                   num_devices=NCORES)
    xts = [nc.dram_tensor(f"xt{j}", [KP, NS * KCH * cols], mm_dt,
                          kind="ExternalInput").ap()
           for j, cols in enumerate(COL_TILES)]
    w1f = nc.dram_tensor("w1f", [KP, NS * KCH * NH], mm_dt,
                         kind="ExternalInput").ap()
    a1 = nc.dram_tensor("a1", [KP, NH], f32, kind="ExternalInput").ap()
    w2a = nc.dram_tensor("w2a", [KP, NOUT], f32, kind="ExternalInput").ap()
    b1 = nc.dram_tensor("b1", [NH, 1], f32, kind="ExternalInput").ap()
    bet = nc.dram_tensor("bet", [NH, 1], f32, kind="ExternalInput").ap()
    s1init = nc.dram_tensor("s1init", [KP, BL + TBL], f32,
                            kind="ExternalInput").ap()
    spk_o = nc.dram_tensor("spk", [NOUT, TBL], f32, kind="ExternalOutput").ap()
    mem_o = nc.dram_tensor("mem", [NOUT, TBL], f32, kind="ExternalOutput").ap()
    aps = (xts, w1f, a1, w2a, b1, bet, s1init, spk_o, mem_o)
    with tile.TileContext(nc) as tc:
        _build_body(tc, nc, mybir, aps, precision)
    nc.compile()
    _PROG[precision] = nc
    return nc


def _mm_layout(kxn, nsplit):
    """[K=NIN, N] fp32 -> [128, nsplit*KCH*N] in matmul-ready order
    (chunk-major, hi|lo interleaved per chunk)."""
    import ml_dtypes
    n = kxn.shape[1]
    v = np.ascontiguousarray(
        kxn.reshape(KCH, KP, n).transpose(1, 0, 2))     # [128, KCH, n]
    if nsplit == 1:
        return v.reshape(KP, KCH * n)
    hi = v.astype(ml_dtypes.bfloat16)
    lo = (v - hi.astype(np.float32)).astype(ml_dtypes.bfloat16)
    out = np.empty((KP, KCH, 2, n), hi.dtype)
    out[:, :, 0, :] = hi
    out[:, :, 1, :] = lo
    return np.ascontiguousarray(out).reshape(KP, 2 * KCH * n)


def prep_inputs(x, W1, b1, beta1, V, Vb, W2, b2, precision=None):
    """Host-side shard + layout prep. Returns list of per-core input dicts."""
    precision = precision or PRECISION
    nsplit = 2 if precision == "bf16x2" else 1
    f32 = np.float32
    w1f = _mm_layout(np.ascontiguousarray(W1.T, dtype=f32), nsplit)
    a1 = np.zeros((KP, NH), f32)
    a1[:NH] = (V - THRESH * np.eye(NH, dtype=f32)).T
    a1[NH] = Vb
    w2a = np.zeros((KP, NOUT), f32)
    w2a[:NH] = W2.T
    w2a[NH] = b2
    b1a = np.ascontiguousarray(b1.reshape(NH, 1), dtype=f32)
    beta = np.clip(beta1, 0.0, 1.0).astype(f32).reshape(NH, 1)
    s1init = np.zeros((KP, BL + TBL), f32)
    s1init[NH] = 1.0
    # x: [T, B, NIN] -> per-core column tiles in matmul-ready layout
    xt_full = np.ascontiguousarray(x.transpose(2, 0, 1))        # [NIN, T, B]
    col_edges = np.cumsum([0] + COL_TILES)
    in_maps = []
    for c in range(NCORES):
        xTc = np.ascontiguousarray(
            xt_full[:, :, c * BL:(c + 1) * BL]).reshape(NIN, TBL)
        m = dict(w1f=w1f, a1=a1, w2a=w2a, b1=b1a, bet=beta, s1init=s1init)
        for j, cols in enumerate(COL_TILES):
            m[f"xt{j}"] = _mm_layout(
                np.ascontiguousarray(xTc[:, col_edges[j]:col_edges[j + 1]]),
                nsplit)
        in_maps.append(m)
    return in_maps


def gather_outputs(results):
    """results: list of per-core {'spk': [NOUT, TBL], 'mem': [NOUT, TBL]}."""
    spks, mems = [], []
    for r in results:
        spks.append(np.ascontiguousarray(
            r["spk"].reshape(NOUT, T, BL).transpose(1, 2, 0)))
        mems.append(np.ascontiguousarray(
            r["mem"].reshape(NOUT, T, BL).transpose(1, 2, 0)))
    spk = np.concatenate(spks, axis=1)
    mem = np.concatenate(mems, axis=1)
    return spk.astype(np.float32), mem.astype(np.float32)


def kernel(x, W1, b1, beta1, V, Vb, W2, b2, **_run_kwargs):
    from concourse import bass_utils

    precision = _run_kwargs.pop("precision", None) or PRECISION
    nc = build_program(precision)
    in_maps = prep_inputs(np.asarray(x, np.float32), np.asarray(W1, np.float32),
                          np.asarray(b1, np.float32), np.asarray(beta1, np.float32),
                          np.asarray(V, np.float32), np.asarray(Vb, np.float32),
                          np.asarray(W2, np.float32), np.asarray(b2, np.float32),
                          precision)
    res = bass_utils.run_bass_kernel_spmd(
        nc, in_maps, core_ids=list(range(NCORES)), **_run_kwargs)
    out = gather_outputs(res.results)
    kernel.last_result = res
    return out
